# revision 3
# baseline (speedup 1.0000x reference)
"""Trainium2 Bass kernel for nn_D_GA_1812476199112 (maxpool -> 16-head
attention over 1024 tokens -> proj -> batchnorm -> maxunpool).

Sharding: data-parallel over batch B=8, one batch element per NeuronCore.
Everything is local per core; no collectives.

Per-core pipeline (channels-on-partitions layout [C=64, N=1024]):
  1. MaxPool2d(2,2) via 3 strided DVE max ops; argmax as first-match masks.
  2. Q^T/K^T computed directly in a "strip-packed" layout (head h of
     supergroup sg at partitions 32c..32c+3) using host-permuted weight
     matrices, so score matmuls can run 3x concurrent via PE row tiling
     (tile_position=(32c,0), K=4).
  3. Scores S^T [keys, queries] land in PSUM chunks [128, 3*512]; one ACT
     Exp instruction per chunk (scale=0.5 folds the softmax scale).
  4. AV matmuls use a ones-augmented V (V~ [128, 5] per head) so softmax
     denominators accumulate in PSUM row 32c+4 for free; col tiling
     (tile_position=(0,32c)) packs 4 heads into one PSUM accumulator.
  5. Tail: one-hot const matmuls gather denominators (em) and reorder the
     o rows (gm) into (h,d) order; DVE reciprocal + multiply normalizes;
     proj matmul; BN folded into one ACT op; unpool via 4 masked DVE
     multiplies writing strided output views.
"""
import numpy as np

DIM = 64
HEAD_DIM = 4
NUM_HEADS = 16
B = 8
H = W = 64
HP = WP = 32
N = HP * WP          # 1024 tokens
NKT = 8              # key tiles of 128
BN_EPS = 1e-5

_CACHE = {}


def _build_program():
    import concourse.bass as bass
    import concourse.mybir as mybir
    import concourse.tile as tile
    from concourse import bacc

    f32 = mybir.dt.float32
    AF = mybir.ActivationFunctionType
    OP = mybir.AluOpType

    nc = bacc.Bacc("TRN2", debug=False)

    x_d = nc.dram_tensor("x", [DIM, H * W], f32, kind="ExternalInput").ap()
    wqp_d = nc.dram_tensor("wqp", [4, DIM, 128], f32, kind="ExternalInput").ap()
    wkp_d = nc.dram_tensor("wkp", [4, DIM, 128], f32, kind="ExternalInput").ap()
    wv_d = nc.dram_tensor("wv", [DIM, DIM], f32, kind="ExternalInput").ap()
    wproj_d = nc.dram_tensor("wproj", [DIM, DIM], f32, kind="ExternalInput").ap()
    bns_d = nc.dram_tensor("bns", [DIM, 1], f32, kind="ExternalInput").ap()
    bnb_d = nc.dram_tensor("bnb", [DIM, 1], f32, kind="ExternalInput").ap()
    em_d = nc.dram_tensor("em", [4, 128, DIM], f32, kind="ExternalInput").ap()
    gm_d = nc.dram_tensor("gm", [4, 128, DIM], f32, kind="ExternalInput").ap()
    out_d = nc.dram_tensor("out", [DIM, H * W], f32, kind="ExternalOutput").ap()

    with tile.TileContext(nc) as tc:
        with (
            tc.tile_pool(name="singles", bufs=1) as sg1,
            tc.tile_pool(name="expp", bufs=3) as expp,
        ):
            # ---------- loads ----------
            x_sb = sg1.tile([DIM, H * W], f32)
            nc.sync.dma_start(out=x_sb, in_=x_d)
            wv_sb = sg1.tile([DIM, DIM], f32)
            nc.sync.dma_start(out=wv_sb, in_=wv_d)
            wproj_sb = sg1.tile([DIM, DIM], f32)
            nc.sync.dma_start(out=wproj_sb, in_=wproj_d)
            bns_sb = sg1.tile([DIM, 1], f32)
            nc.sync.dma_start(out=bns_sb, in_=bns_d)
            bnb_sb = sg1.tile([DIM, 1], f32)
            nc.sync.dma_start(out=bnb_sb, in_=bnb_d)
            wqp_sb, wkp_sb, em_sb, gm_sb = [], [], [], []
            for sg in range(4):
                t = sg1.tile([DIM, 128], f32, tag=f"wqp{sg}")
                nc.sync.dma_start(out=t, in_=wqp_d[sg])
                wqp_sb.append(t)
                t = sg1.tile([DIM, 128], f32, tag=f"wkp{sg}")
                nc.sync.dma_start(out=t, in_=wkp_d[sg])
                wkp_sb.append(t)
                t = sg1.tile([128, DIM], f32, tag=f"em{sg}")
                nc.sync.dma_start(out=t, in_=em_d[sg])
                em_sb.append(t)
                t = sg1.tile([128, DIM], f32, tag=f"gm{sg}")
                nc.sync.dma_start(out=t, in_=gm_d[sg])
                gm_sb.append(t)

            # ---------- maxpool ----------
            xr = x_sb.rearrange("p (i ti j tj) -> p i ti j tj", ti=2, tj=2, j=WP)
            v = [xr[:, :, 0, :, 0], xr[:, :, 0, :, 1],
                 xr[:, :, 1, :, 0], xr[:, :, 1, :, 1]]
            m01 = sg1.tile([DIM, N], f32)
            m23 = sg1.tile([DIM, N], f32)
            pooled = sg1.tile([DIM, N], f32)
            m01r = m01.rearrange("p (i j) -> p i j", j=WP)
            m23r = m23.rearrange("p (i j) -> p i j", j=WP)
            pooledr = pooled.rearrange("p (i j) -> p i j", j=WP)
            nc.vector.tensor_tensor(m01r, v[0], v[1], op=OP.max)
            nc.vector.tensor_tensor(m23r, v[2], v[3], op=OP.max)
            nc.vector.tensor_tensor(pooled, m01, m23, op=OP.max)

            # ---------- argmax masks (first-match) ----------
            masks = []
            nf = None
            for p in range(4):
                eq = sg1.tile([DIM, N], f32, tag=f"eq{p}")
                eqr = eq.rearrange("p (i j) -> p i j", j=WP)
                nc.vector.tensor_tensor(eqr, v[p], pooledr, op=OP.is_equal)
                if p == 0:
                    masks.append(eq)
                    nf = sg1.tile([DIM, N], f32, tag="nf0")
                    nc.vector.tensor_scalar(nf, eq, -1.0, 1.0, op0=OP.mult, op1=OP.add)
                else:
                    mk = sg1.tile([DIM, N], f32, tag=f"mk{p}")
                    nc.vector.tensor_tensor(mk, eq, nf, op=OP.mult)
                    masks.append(mk)
                    if p < 3:
                        nf2 = sg1.tile([DIM, N], f32, tag=f"nf{p}")
                        nc.vector.tensor_tensor(nf2, nf, mk, op=OP.subtract)
                        nf = nf2

            # ---------- qkv packs + V~ ----------
            qtp, ktp = [], []
            with tc.tile_pool(name="prepps", bufs=2, space="PSUM") as prepps:
                for sg in range(4):
                    qt_ps = prepps.tile([128, N], f32, tag="qkps")
                    for qh in range(2):
                        nc.tensor.matmul(
                            qt_ps[:, qh * 512:(qh + 1) * 512],
                            wqp_sb[sg], pooled[:, qh * 512:(qh + 1) * 512],
                            start=True, stop=True)
                    t = sg1.tile([128, N], f32, tag=f"qtp{sg}")
                    nc.vector.tensor_copy(t, qt_ps)
                    qtp.append(t)
                    kt_ps = prepps.tile([128, N], f32, tag="qkps")
                    for qh in range(2):
                        nc.tensor.matmul(
                            kt_ps[:, qh * 512:(qh + 1) * 512],
                            wkp_sb[sg], pooled[:, qh * 512:(qh + 1) * 512],
                            start=True, stop=True)
                    t = sg1.tile([128, N], f32, tag=f"ktp{sg}")
                    nc.vector.tensor_copy(t, kt_ps)
                    ktp.append(t)
                vt = []
                for kt in range(NKT):
                    v_ps = prepps.tile([128, DIM], f32, tag="vps")
                    nc.tensor.matmul(
                        v_ps, pooled[:, kt * 128:(kt + 1) * 128], wv_sb,
                        start=True, stop=True)
                    vtile = sg1.tile([128, 16, 5], f32, tag=f"vt{kt}")
                    nc.vector.tensor_copy(
                        vtile[:, :, 0:4],
                        v_ps.rearrange("p (h e) -> p h e", e=4))
                    nc.vector.memset(vtile[:, :, 4:5], 1.0)
                    vt.append(vtile)

            # ---------- attention ----------
            chunks = [
                [(0, 0), (1, 0), (2, 0)],
                [(3, 0), (0, 1), (1, 1)],
                [(2, 1), (3, 1)],
            ]
            o_sb = []
            with (
                tc.tile_pool(name="spsum", bufs=2, space="PSUM") as spsum,
                tc.tile_pool(name="opsum", bufs=1, space="PSUM") as opsum,
            ):
                for sg in range(4):
                    o_ps = opsum.tile([128, N], f32, tag="ops")
                    for kt in range(NKT):
                        for ch in chunks:
                            ncb = len(ch)
                            s_ps = spsum.tile([128, 3 * 512], f32, tag="slot")
                            for i, (c, qh) in enumerate(ch):
                                nc.tensor.matmul(
                                    s_ps[:, i * 512:(i + 1) * 512],
                                    ktp[sg][32 * c:32 * c + 4,
                                            kt * 128:(kt + 1) * 128],
                                    qtp[sg][32 * c:32 * c + 4,
                                            qh * 512:(qh + 1) * 512],
                                    start=True, stop=True,
                                    tile_position=(32 * c, 0))
                            e_sb = expp.tile([128, 3 * 512], f32, tag="exp")
                            nc.scalar.activation(
                                e_sb[:, :ncb * 512], s_ps[:, :ncb * 512],
                                AF.Exp, scale=0.5)
                            for i, (c, qh) in enumerate(ch):
                                nc.tensor.matmul(
                                    o_ps[32 * c:32 * c + 5,
                                         qh * 512:(qh + 1) * 512],
                                    vt[kt][:, 4 * sg + c, :],
                                    e_sb[:, i * 512:(i + 1) * 512],
                                    start=(kt == 0), stop=(kt == NKT - 1),
                                    tile_position=(0, 32 * c))
                    osg = sg1.tile([128, N], f32, tag=f"osb{sg}")
                    nc.vector.memset(osg, 0.0)
                    for c in range(4):
                        nc.vector.tensor_copy(
                            osg[32 * c:32 * c + 5, :], o_ps[32 * c:32 * c + 5, :])
                    o_sb.append(osg)

            # ---------- tail: normalize + proj + bn + unpool ----------
            with tc.tile_pool(name="tailps", bufs=1, space="PSUM") as tailps:
                d_ps = tailps.tile([DIM, N], f32, tag="dps")
                for sg in range(4):
                    nc.tensor.matmul(
                        d_ps[:, 0:512], em_sb[sg], o_sb[sg][:, 0:512],
                        start=(sg == 0), stop=(sg == 3))
                    nc.tensor.matmul(
                        d_ps[:, 512:1024], em_sb[sg], o_sb[sg][:, 512:1024],
                        start=(sg == 0), stop=(sg == 3))
                dr = sg1.tile([DIM, N], f32)
                nc.vector.reciprocal(dr, d_ps)
                o2_ps = tailps.tile([DIM, N], f32, tag="o2ps")
                for sg in range(4):
                    nc.tensor.matmul(
                        o2_ps[:, 0:512], gm_sb[sg], o_sb[sg][:, 0:512],
                        start=(sg == 0), stop=(sg == 3))
                    nc.tensor.matmul(
                        o2_ps[:, 512:1024], gm_sb[sg], o_sb[sg][:, 512:1024],
                        start=(sg == 0), stop=(sg == 3))
                onorm = sg1.tile([DIM, N], f32)
                nc.vector.tensor_tensor(onorm, o2_ps, dr, op=OP.mult)
                pj_ps = tailps.tile([DIM, N], f32, tag="pjps")
                for qh in range(2):
                    nc.tensor.matmul(
                        pj_ps[:, qh * 512:(qh + 1) * 512],
                        wproj_sb, onorm[:, qh * 512:(qh + 1) * 512],
                        start=True, stop=True)
                y = sg1.tile([DIM, N], f32)
                nc.scalar.activation(
                    y, pj_ps, AF.Identity, bias=bnb_sb, scale=bns_sb)

            out_sb = sg1.tile([DIM, H * W], f32)
            outr = out_sb.rearrange("p (i ti j tj) -> p i ti j tj",
                                    ti=2, tj=2, j=WP)
            yr = y.rearrange("p (i j) -> p i j", j=WP)
            for p in range(4):
                mr = masks[p].rearrange("p (i j) -> p i j", j=WP)
                nc.vector.tensor_tensor(
                    outr[:, :, p // 2, :, p % 2], yr, mr, op=OP.mult)
            nc.sync.dma_start(out=out_d, in_=out_sb)

    nc.compile()
    return nc


def _host_inputs(x, w_qkv, w_proj, gamma, beta, bn_mean, bn_var):
    """Build the per-core input maps (host-side packing)."""
    wq = w_qkv[:, 0:64]
    wk = w_qkv[:, 64:128]
    wv = np.ascontiguousarray(w_qkv[:, 128:192], dtype=np.float32)
    wqp = np.zeros((4, DIM, 128), np.float32)
    wkp = np.zeros((4, DIM, 128), np.float32)
    em = np.zeros((4, 128, DIM), np.float32)
    gm = np.zeros((4, 128, DIM), np.float32)
    for sg in range(4):
        for c in range(4):
            h = 4 * sg + c
            for d in range(HEAD_DIM):
                wqp[sg][:, 32 * c + d] = wq[:, 4 * h + d]
                wkp[sg][:, 32 * c + d] = wk[:, 4 * h + d]
                gm[sg][32 * c + d, 4 * h + d] = 1.0
                em[sg][32 * c + 4, 4 * h + d] = 1.0
    inv = gamma / np.sqrt(bn_var + BN_EPS)
    bns = inv.reshape(DIM, 1).astype(np.float32)
    bnb = (beta - bn_mean * inv).reshape(DIM, 1).astype(np.float32)
    wproj = np.ascontiguousarray(w_proj, dtype=np.float32)

    shared = {"wqp": wqp, "wkp": wkp, "wv": wv, "wproj": wproj,
              "bns": bns, "bnb": bnb, "em": em, "gm": gm}
    in_maps = []
    for b in range(B):
        m = dict(shared)
        m["x"] = np.ascontiguousarray(
            np.asarray(x)[b].reshape(DIM, H * W), dtype=np.float32)
        in_maps.append(m)
    return in_maps


def kernel(x, w_qkv, w_proj, gamma, beta, bn_mean, bn_var):
    from concourse import bass_utils

    if "nc" not in _CACHE:
        _CACHE["nc"] = _build_program()
    nc = _CACHE["nc"]
    in_maps = _host_inputs(
        np.asarray(x), np.asarray(w_qkv), np.asarray(w_proj),
        np.asarray(gamma), np.asarray(beta),
        np.asarray(bn_mean), np.asarray(bn_var))
    res = bass_utils.run_bass_kernel_spmd(nc, in_maps, core_ids=list(range(B)))
    out = np.stack([res.results[b]["out"].reshape(DIM, H, W) for b in range(B)])
    return out.astype(np.float32)
